# revision 1
# baseline (speedup 1.0000x reference)
"""Multi-head self-attention with positional bias, sharded over 8 NeuronCores.

Sharding: head-parallel. Core h computes head h for all batches:
  q/k/v projections with the head's weight slices, scores + softmax with the
  head's pos_bias slice, and the partial output  o_h @ Wout[h*64:(h+1)*64, :].
The full output is the sum of the 8 partials (row-parallel Wout).

Device kernel math (per core):
  - query is supplied pre-transposed (qT [D, B*N]) so the contraction dim of
    every projection lands on SBUF partitions.
  - scores are computed TRANSPOSED: ST[j, i] = bias[i, j] + k_j . q_i, so exp's
    output P~[j, i] is directly the layout the attention*V matmul needs (no P
    transposes). The bias lands in PSUM via an identity matmul (start=True),
    the qk matmul accumulates on top.
  - exp skips max-subtraction: scores are ~N(0, 2) (bounded), exp is safe in
    fp32 and softmax is shift-invariant.
  - softmax denominator: ones column appended to v (safe mode) or packed
    ones-matmuls (packed mode); normalization is deferred to the PSUM
    evacuation after the Wout matmul (per-partition scalar multiply).
  - all matmuls run in float32r (fp22 mantissa truncation, full PE speed at
    free-dim >= 256, fp32 accumulation): rel err ~1e-4.
"""

import numpy as np
from contextlib import ExitStack

import concourse.bass as bass
import concourse.bacc as bacc
import concourse.mybir as mybir
import concourse.tile as tile
from concourse.bass_utils import run_bass_kernel_spmd
from concourse.masks import make_identity

HEADS = 8
DH = 64
B, N, D = 4, 2048, 512
SCALE = DH ** -0.5
N_CORES = 8
PACKED = False  # shared-PSUM-bank packing tricks (col-strip oT, 4-way denom)

F32 = mybir.dt.float32
F32R = mybir.dt.float32r


def build_nc(b=B, n=N, d=D, packed=PACKED, n_cores=1):
    """Build the per-core Bass program. All cores run the same program (SPMD);
    per-head differences come in through the input tensors."""
    assert b % 2 == 0 and n % 512 == 0 and d % 128 == 0
    T = b * n           # total tokens
    CC = d // 128       # contraction chunks for the projections
    NJ = n // 128       # key tiles (j)
    NIC = n // 512      # query chunks of 512 (i)
    assert NIC % 2 == 0
    NIP = NIC // 2      # i-groups of 1024 (one exp op each)
    NPAIR = b // 2
    IC = 512
    VW = 64 if packed else 65  # v block width (safe mode: +1 ones column)

    nc = bacc.Bacc("TRN2", target_bir_lowering=False, debug=False,
                   num_devices=n_cores)
    qT = nc.declare_dram_parameter("qT", [d, T], F32R, isOutput=False)
    biasT = nc.declare_dram_parameter("biasT", [n, n], F32R, isOutput=False)
    wq = nc.declare_dram_parameter("wq", [d, DH], F32R, isOutput=False)
    wk = nc.declare_dram_parameter("wk", [d, DH], F32R, isOutput=False)
    wv = nc.declare_dram_parameter("wv", [d, DH], F32R, isOutput=False)
    wout = nc.declare_dram_parameter("wout", [DH, d], F32R, isOutput=False)
    out = nc.declare_dram_parameter("out", [T, d], F32, isOutput=True)

    with ExitStack() as ctx:
        tc = ctx.enter_context(tile.TileContext(nc))

        const = ctx.enter_context(tc.tile_pool(name="const", bufs=1))
        qk_pool = ctx.enter_context(tc.tile_pool(name="qkT", bufs=1))
        v_pool = ctx.enter_context(tc.tile_pool(name="v", bufs=1))
        ot_sb_pool = ctx.enter_context(tc.tile_pool(name="ot_sb", bufs=1))
        p_pool = ctx.enter_context(tc.tile_pool(name="pexp", bufs=4))
        out_pool = ctx.enter_context(tc.tile_pool(name="osb", bufs=6))

        ident_f32 = const.tile([128, 128], F32, tag="ident_f32")
        make_identity(nc, ident_f32)
        ident = const.tile([128, 128], F32R, tag="ident")
        nc.vector.tensor_copy(ident, ident_f32)
        zbias = const.tile([128, 1], F32, tag="zbias")
        nc.vector.memset(zbias, 0.0)
        ones16 = const.tile([128, 16], F32, tag="ones16")
        nc.vector.memset(ones16, 1.0)
        if packed:
            ones32 = const.tile([128, 32], F32R, tag="ones32")
            nc.vector.tensor_copy(ones32[:, 0:16], ones16)
            nc.vector.tensor_copy(ones32[:, 16:32], ones16)

        w_sb = {}
        for name, w in (("wq", wq), ("wk", wk), ("wv", wv)):
            t = const.tile([128, CC, DH], F32R, tag=name)
            nc.sync.dma_start(out=t, in_=w[:, :].rearrange("(c p) e -> p c e", p=128))
            w_sb[name] = t
        wout_sb = const.tile([128, d], F32R, tag="wout")
        nc.sync.dma_start(out=wout_sb[0:64, :], in_=wout[:, :])
        nc.sync.dma_start(out=wout_sb[64:128, :], in_=wout[:, :])

        qT_sb = [qk_pool.tile([128, n], F32R, tag=f"qT{p}", name=f"qT{p}") for p in range(NPAIR)]
        kT_sb = [qk_pool.tile([128, n], F32R, tag=f"kT{p}", name=f"kT{p}") for p in range(NPAIR)]
        v_sb = [v_pool.tile([128, NJ * VW], F32R, tag=f"v{bb}", name=f"v{bb}") for bb in range(b)]
        if not packed:
            for bb in range(b):
                ones_cols = v_sb[bb].rearrange("p (t w) -> p t w", w=VW)[:, :, DH:VW]
                nc.vector.tensor_copy(ones_cols, ones16[:, 0:NJ].rearrange("p (t o) -> p t o", o=1))
        ot_sb = [ot_sb_pool.tile([128, n], F32R, tag=f"ot{p}", name=f"ot{p}") for p in range(NPAIR)]

        # denominator staging: row bb lives at partition 32*bb (engines need
        # 32-aligned partition bases)
        den_all = const.tile([32 * (b - 1) + 1, n], F32, tag="den_all")
        den_sb = [den_all[32 * bb:32 * bb + 1, :] for bb in range(b)]
        recip_in = [const.tile([128, NJ], F32, tag=f"recip_in{bb}", name=f"ri{bb}")
                    for bb in range(b)]
        recip_sb = [const.tile([128, NJ], F32, tag=f"recip_sb{bb}", name=f"rs{bb}")
                    for bb in range(b)]

        # ---------------- projections (per batch) ----------------
        HN = max(n // 4, 512)  # qt chunk width (>= one projection rhs slice)
        NQ = n // HN
        with tc.tile_pool(name="qt", bufs=3 * CC) as qt_pool, \
             tc.tile_pool(name="pqk", bufs=4, space="PSUM") as pqk_pool, \
             tc.tile_pool(name="pv", bufs=4, space="PSUM") as pv_pool:
            for bb in range(b):
                pair, lb = bb // 2, bb % 2
                rows = slice(64 * lb, 64 * lb + 64)
                for hh in range(NQ):
                    qt_c = []
                    for c in range(CC):
                        t = qt_pool.tile([128, HN], F32R, tag="qt", name="qtc")
                        nc.sync.dma_start(
                            out=t, in_=qT[c * 128:(c + 1) * 128,
                                          bb * n + hh * HN: bb * n + (hh + 1) * HN])
                        qt_c.append(t)
                    for wname, dest in (("wq", qT_sb[pair]), ("wk", kT_sb[pair])):
                        for hic in range(HN // IC):
                            icc = (hh * HN + hic * IC) // IC
                            ps = pqk_pool.tile([64, IC], F32, tag="pqk")
                            for c in range(CC):
                                nc.tensor.matmul(
                                    ps, lhsT=w_sb[wname][:, c, :],
                                    rhs=qt_c[c][:, hic * IC:(hic + 1) * IC],
                                    start=(c == 0), stop=(c == CC - 1))
                            nc.vector.tensor_copy(dest[rows, icc * IC:(icc + 1) * IC], ps)
                    for htt in range(HN // 128):
                        tt = (hh * HN + htt * 128) // 128
                        psv = pv_pool.tile([128, DH], F32, tag="pv")
                        for c in range(CC):
                            nc.tensor.matmul(
                                psv, lhsT=qt_c[c][:, htt * 128:(htt + 1) * 128],
                                rhs=w_sb["wv"][:, c, :],
                                start=(c == 0), stop=(c == CC - 1))
                        nc.vector.tensor_copy(v_sb[bb][:, tt * VW: tt * VW + DH], psv)

        # ---------------- scores + softmax + P~^T V ----------------
        with tc.tile_pool(name="bias", bufs=NJ) as bias_pool, \
             tc.tile_pool(name="st", bufs=2, space="PSUM") as st_pool, \
             tc.tile_pool(name="ot", bufs=2 if packed else 4, space="PSUM") as ot_pool, \
             tc.tile_pool(name="dn", bufs=1, space="PSUM") as dn_pool:
            for ip in range(NIP):
                bias_t = []
                for jt in range(NJ):
                    t = bias_pool.tile([128, 2 * IC], F32R, tag="bias")
                    nc.sync.dma_start(
                        out=t, in_=biasT[jt * 128:(jt + 1) * 128, ip * 2 * IC:(ip + 1) * 2 * IC])
                    bias_t.append(t)
                for pair in range(NPAIR):
                    if packed:
                        ot_ps = [ot_pool.tile([128, IC], F32, tag="ot", name="otp")
                                 for _ in range(2)]
                        dn_ps = dn_pool.tile([128, IC], F32, tag="dn")
                    else:
                        ot_ps = {(lb, il): ot_pool.tile([65, IC], F32, tag="ot", name="otp")
                                 for lb in range(2) for il in range(2)}
                    for jt in range(NJ):
                        for lb in range(2):
                            bb = 2 * pair + lb
                            rows = slice(64 * lb, 64 * lb + 64)
                            st = st_pool.tile([128, 2 * IC], F32, tag="st")
                            for il in range(2):
                                cols = slice(il * IC, (il + 1) * IC)
                                ic = ip * 2 + il
                                nc.tensor.matmul(
                                    st[:, cols], lhsT=ident, rhs=bias_t[jt][:, cols],
                                    start=True, stop=False)
                                nc.tensor.matmul(
                                    st[:, cols],
                                    lhsT=kT_sb[pair][rows, jt * 128:(jt + 1) * 128],
                                    rhs=qT_sb[pair][rows, ic * IC:(ic + 1) * IC],
                                    start=False, stop=True)
                            pexp = p_pool.tile([128, 2 * IC], F32R, tag="pexp")
                            nc.scalar.activation(
                                pexp, st, mybir.ActivationFunctionType.Exp, bias=zbias)
                            for il in range(2):
                                pcols = slice(il * IC, (il + 1) * IC)
                                if packed:
                                    nc.tensor.matmul(
                                        ot_ps[il][rows, :],
                                        lhsT=v_sb[bb][:, jt * VW: jt * VW + DH],
                                        rhs=pexp[:, pcols],
                                        start=(jt == 0 and lb == 0),
                                        stop=(jt == NJ - 1 and lb == 1),
                                        skip_group_check=True)
                                    s_idx = il * 2 + lb
                                    nc.tensor.matmul(
                                        dn_ps[32 * s_idx: 32 * s_idx + 32, :],
                                        lhsT=ones32, rhs=pexp[:, pcols],
                                        start=(jt == 0 and s_idx == 0),
                                        stop=(jt == NJ - 1 and s_idx == 3),
                                        tile_position=(0, 32 * s_idx),
                                        skip_group_check=True)
                                else:
                                    nc.tensor.matmul(
                                        ot_ps[(lb, il)],
                                        lhsT=v_sb[bb][:, jt * VW: jt * VW + VW],
                                        rhs=pexp[:, pcols],
                                        start=(jt == 0), stop=(jt == NJ - 1))
                    # evacuate oT + denominators for this (ip, pair)
                    for il in range(2):
                        ic = ip * 2 + il
                        ccols = slice(ic * IC, (ic + 1) * IC)
                        if packed:
                            for lb in range(2):
                                s_idx = il * 2 + lb
                                bb = 2 * pair + lb
                                nc.vector.tensor_copy(
                                    den_sb[bb][0:1, ccols],
                                    dn_ps[32 * s_idx: 32 * s_idx + 1, :])
                            nc.vector.tensor_copy(ot_sb[pair][:, ccols], ot_ps[il])
                        else:
                            for lb in range(2):
                                bb = 2 * pair + lb
                                rows = slice(64 * lb, 64 * lb + 64)
                                nc.vector.tensor_copy(
                                    den_sb[bb][0:1, ccols], ot_ps[(lb, il)][64:65, :])
                                nc.vector.tensor_copy(
                                    ot_sb[pair][rows, ccols], ot_ps[(lb, il)][0:64, :])

        # denominator rows -> per-token-tile columns (via DRAM bounce), reciprocal
        for bb in range(b):
            den_dram = nc.dram_tensor(f"den_dram{bb}", [n], F32)
            nc.sync.dma_start(out=den_dram[:], in_=den_sb[bb][0:1, :])
            nc.sync.dma_start(
                out=recip_in[bb],
                in_=den_dram[:].rearrange("(t p) -> p t", p=128))
            nc.vector.reciprocal(recip_sb[bb], recip_in[bb])

        # ---------------- output projection ----------------
        with tc.tile_pool(name="po", bufs=6, space="PSUM") as po_pool:
            for pair in range(NPAIR):
                for tg in range(NJ):
                    for lb in range(2):
                        bb = 2 * pair + lb
                        rows = slice(64 * lb, 64 * lb + 64)
                        po = po_pool.tile([128, d], F32, tag="po")
                        nc.tensor.matmul(
                            po, lhsT=ot_sb[pair][rows, tg * 128:(tg + 1) * 128],
                            rhs=wout_sb[rows, :], start=True, stop=True)
                        osb = out_pool.tile([128, d], F32, tag="osb")
                        nc.vector.tensor_scalar_mul(
                            osb, po, recip_sb[bb][:, tg: tg + 1])
                        nc.sync.dma_start(
                            out=out[bb * n + tg * 128: bb * n + (tg + 1) * 128, :],
                            in_=osb)
    nc.compile()
    return nc


def make_in_maps(query, pos_bias, Wq, Wk, Wv, Wout, n_cores=N_CORES):
    """Host-side sharding/layout prep. Head h -> core h."""
    query = np.asarray(query, dtype=np.float32)
    pos_bias = np.asarray(pos_bias, dtype=np.float32)
    Wq = np.asarray(Wq, dtype=np.float32)
    Wk = np.asarray(Wk, dtype=np.float32)
    Wv = np.asarray(Wv, dtype=np.float32)
    Wout = np.asarray(Wout, dtype=np.float32)

    b, n, d = query.shape
    qT = np.ascontiguousarray(query.reshape(b * n, d).T)
    wq_s = Wq * np.float32(SCALE)
    in_maps = []
    for h in range(n_cores):
        sl = slice(h * DH, (h + 1) * DH)
        in_maps.append({
            "qT": qT,
            "biasT": np.ascontiguousarray(pos_bias[h].T),
            "wq": np.ascontiguousarray(wq_s[:, sl]),
            "wk": np.ascontiguousarray(Wk[:, sl]),
            "wv": np.ascontiguousarray(Wv[:, sl]),
            "wout": np.ascontiguousarray(Wout[sl, :]),
        })
    return in_maps


def run_device(in_maps, b=B, n=N, d=D, packed=PACKED, trace=False, **kw):
    nc = build_nc(b, n, d, packed, n_cores=len(in_maps))
    return run_bass_kernel_spmd(nc, in_maps, list(range(len(in_maps))), trace=trace, **kw)


def assemble(results, b=B, n=N, d=D):
    acc = np.zeros((b * n, d), dtype=np.float32)
    for r in results:
        acc += r["out"]
    return acc.reshape(b, n, d)


def kernel(query, pos_bias, Wq, Wk, Wv, Wout):
    in_maps = make_in_maps(query, pos_bias, Wq, Wk, Wv, Wout)
    res = run_device(in_maps)
    return assemble(res.results)



# revision 9
# speedup vs baseline: 1.0610x; 1.0610x over previous
"""Multi-head self-attention with positional bias, sharded over 8 NeuronCores.

Sharding: head-parallel. Core h computes head h for all batches; the full
output is the sum of the 8 per-core partials (row-parallel Wout), reduced on
host.

Device kernel (per core), fp16 matmul inputs / fp32 PSUM accumulation:
  - projections: packed q|k weight [d, 128] gives one [128, 512]-psum chain
    per token chunk (q rows 0-63, k rows 64-127); v accumulates 16 token
    tiles side by side in one [128, 1024] psum tile.
  - scores are computed TRANSPOSED: ST[j, i] = k_j . q_i so exp's output is
    directly the layout the attention*V matmul needs.
  - the positional bias never touches the PE: host ships E = exp(bias^T) and
    the device computes P~ = exp(ST) * E with a 2x-mode fp16 DVE multiply.
  - softmax denominator: ones column appended to v; PV matmul row 64 then
    holds sum_j P~[j, i]. Normalization happens BEFORE the output projection
    (ot * recip[i], a broadcast fp16 multiply), so the Wout psum tiles DMA
    straight to DRAM with no extra engine pass.
"""

import numpy as np
from contextlib import ExitStack

import concourse.bass as bass
import concourse.bacc as bacc
import concourse.mybir as mybir
import concourse.tile as tile
from concourse.bass_utils import run_bass_kernel_spmd

HEADS = 8
DH = 64
B, N, D = 4, 2048, 512
SCALE = DH ** -0.5
N_CORES = 8

F32 = mybir.dt.float32
F16 = mybir.dt.float16
MUL = mybir.AluOpType.mult


def build_nc(b=B, n=N, d=D, n_cores=1):
    """Per-core Bass program (SPMD: per-head differences come in via inputs)."""
    assert b % 2 == 0 and n % 512 == 0 and d % 128 == 0
    T = b * n
    CC = d // 128        # contraction chunks for projections
    NJ = n // 128        # key tiles (j)
    IC = 512
    NIC = n // IC        # i-chunks of 512
    NIP = NIC // 2       # i-groups of 1024
    NPAIR = b // 2
    VW = DH + 1          # v block width (+1 ones column for denominator)

    nc = bacc.Bacc("TRN2", target_bir_lowering=False, debug=False,
                   num_devices=n_cores)
    qT = nc.declare_dram_parameter("qT", [d, T], F16, isOutput=False)
    eb = nc.declare_dram_parameter("eb", [n, n], F16, isOutput=False)
    wqk = nc.declare_dram_parameter("wqk", [d, 2 * DH], F16, isOutput=False)
    wv = nc.declare_dram_parameter("wv", [d, DH], F16, isOutput=False)
    wout = nc.declare_dram_parameter("wout", [DH, d], F16, isOutput=False)
    out = nc.declare_dram_parameter("out", [T, d], F16, isOutput=True)

    with ExitStack() as ctx:
        tc = ctx.enter_context(tile.TileContext(nc))

        const = ctx.enter_context(tc.tile_pool(name="const", bufs=1))
        qk_pool = ctx.enter_context(tc.tile_pool(name="qkT", bufs=1))
        v_pool = ctx.enter_context(tc.tile_pool(name="v", bufs=1))
        e_pool = ctx.enter_context(tc.tile_pool(name="ebias", bufs=1))
        ot_pool = ctx.enter_context(tc.tile_pool(name="otf", bufs=3))
        qt_pool = ctx.enter_context(tc.tile_pool(name="qt", bufs=6))
        p_pool = ctx.enter_context(tc.tile_pool(name="pexp", bufs=4))
        pr_pool = ctx.enter_context(tc.tile_pool(name="prod", bufs=4))
        osb_pool = ctx.enter_context(tc.tile_pool(name="osb", bufs=4))
        # PSUM: st_pool holds score tiles, projection accumulators and output
        # po tiles (all [128, 1024] f32 = 2 banks); ots holds PV accumulators.
        st_pool = ctx.enter_context(tc.tile_pool(name="st", bufs=2, space="PSUM"))
        ots_pool = ctx.enter_context(tc.tile_pool(name="ots", bufs=4, space="PSUM"))

        zbias = const.tile([128, 1], F32, tag="zbias")
        nc.vector.memset(zbias, 0.0)
        ones16 = const.tile([128, 16], F16, tag="ones16")
        nc.vector.memset(ones16, 1.0)

        wqk_sb = const.tile([128, CC, 2 * DH], F16, tag="wqk")
        nc.sync.dma_start(out=wqk_sb, in_=wqk[:, :].rearrange("(c p) e -> p c e", p=128))
        wv_sb = const.tile([128, CC, DH], F16, tag="wv")
        nc.sync.dma_start(out=wv_sb, in_=wv[:, :].rearrange("(c p) e -> p c e", p=128))
        wout_sb = const.tile([DH, d], F16, tag="wout")
        nc.sync.dma_start(out=wout_sb, in_=wout[:, :])

        # exp(bias^T), staged whole: E_sb[jt][p, i] = exp(bias[i, jt*128+p])
        e_sb = []
        for jt in range(NJ):
            t = e_pool.tile([128, n], F16, tag=f"eb{jt}", name=f"eb{jt}")
            nc.sync.dma_start(out=t, in_=eb[jt * 128:(jt + 1) * 128, :])
            e_sb.append(t)

        qT_sb = [qk_pool.tile([DH, n], F16, tag=f"qT{bb}", name=f"qT{bb}") for bb in range(b)]
        kT_sb = [qk_pool.tile([DH, n], F16, tag=f"kT{bb}", name=f"kT{bb}") for bb in range(b)]
        v_sb = [v_pool.tile([128, NJ * VW], F16, tag=f"v{bb}", name=f"v{bb}") for bb in range(b)]
        for bb in range(b):
            ones_cols = v_sb[bb].rearrange("p (t w) -> p t w", w=VW)[:, :, DH:VW]
            nc.vector.tensor_copy(ones_cols, ones16[:, 0:NJ].rearrange("p (t o) -> p t o", o=1))



        # ---------------- projections (per batch) ----------------
        for bb in range(b):
            qt_c = []
            for c in range(CC):
                t = qt_pool.tile([128, n], F16, tag="qt", name=f"qt{bb}_{c}")
                nc.sync.dma_start(out=t, in_=qT[c * 128:(c + 1) * 128, bb * n:(bb + 1) * n])
                qt_c.append(t)
            # q|k packed: psum rows 0-63 = q^T, 64-127 = k^T
            for hh in range(n // (2 * IC)):
                ps = st_pool.tile([128, 2 * IC], F32, tag="st", name=f"pqk{bb}_{hh}")
                for half in range(2):
                    cols = slice(half * IC, (half + 1) * IC)
                    acols = slice(hh * 2 * IC + half * IC, hh * 2 * IC + (half + 1) * IC)
                    for c in range(CC):
                        nc.tensor.matmul(ps[:, cols], lhsT=wqk_sb[:, c, :],
                                         rhs=qt_c[c][:, acols],
                                         start=(c == 0), stop=(c == CC - 1),
                                         skip_group_check=True)
                dcols = slice(hh * 2 * IC, (hh + 1) * 2 * IC)
                nc.vector.tensor_copy(qT_sb[bb][:, dcols], ps[0:DH, :])
                nc.vector.tensor_copy(kT_sb[bb][:, dcols], ps[DH:128, :])
            # v: 16 token tiles side by side in one [128, 1024] psum tile
            psv = st_pool.tile([128, 2 * IC], F32, tag="st", name=f"pv{bb}")
            for tt in range(NJ):
                for c in range(CC):
                    nc.tensor.matmul(psv[:, tt * DH:(tt + 1) * DH],
                                     lhsT=qt_c[c][:, tt * 128:(tt + 1) * 128],
                                     rhs=wv_sb[:, c, :],
                                     start=(c == 0), stop=(c == CC - 1),
                                     skip_group_check=True)
            vdst = v_sb[bb].rearrange("p (t w) -> p t w", w=VW)[:, :, 0:DH]
            nc.vector.tensor_copy(vdst, psv.rearrange("p (t e) -> p t e", e=DH))

        # ---------------- scores + softmax + P~^T V + out-proj ----------------
        exp_fn = mybir.ActivationFunctionType.Exp
        PROD_POOL_JTS = frozenset(range(5, NJ, 5))  # DVE/Pool work split
        for ip in range(NIP):
            for pair in range(NPAIR):
                ot_ps = {(lb, il): ots_pool.tile([VW, IC], F32, tag="ot", name="otp")
                         for lb in range(2) for il in range(2)}
                for jt in range(NJ):
                    for lb in range(2):
                        bb = 2 * pair + lb
                        st = st_pool.tile([128, 2 * IC], F32, tag="st", name="st")
                        for il in range(2):
                            ic = ip * 2 + il
                            nc.tensor.matmul(
                                st[:, il * IC:(il + 1) * IC],
                                lhsT=kT_sb[bb][:, jt * 128:(jt + 1) * 128],
                                rhs=qT_sb[bb][:, ic * IC:(ic + 1) * IC],
                                start=True, stop=True, skip_group_check=True)
                        pexp = p_pool.tile([128, 2 * IC], F16, tag="pexp")
                        nc.scalar.activation(pexp, st, exp_fn, bias=zbias)
                        prod = pr_pool.tile([128, 2 * IC], F16, tag="prod")
                        peng = nc.gpsimd if jt in PROD_POOL_JTS else nc.vector
                        peng.tensor_tensor(
                            prod, pexp, e_sb[jt][:, ip * 2 * IC:(ip + 1) * 2 * IC], MUL)
                        for il in range(2):
                            nc.tensor.matmul(
                                ot_ps[(lb, il)],
                                lhsT=v_sb[bb][:, jt * VW:jt * VW + VW],
                                rhs=prod[:, il * IC:(il + 1) * IC],
                                start=(jt == 0), stop=(jt == NJ - 1))
                # evacuate + reciprocal + normalize + output projection
                for lb in range(2):
                    for il in range(2):
                        bb = 2 * pair + lb
                        ic = ip * 2 + il
                        of = ot_pool.tile([VW, IC], F16, tag="of", name="of")
                        nc.vector.tensor_copy(of, ot_ps[(lb, il)])
                        rr = ot_pool.tile([1, IC], F16, tag="rr", name="rr")
                        with nc.allow_low_precision("fp16 softmax denom reciprocal"):
                            nc.vector.reciprocal(rr, of[DH:VW, :])
                        rbc = ot_pool.tile([DH, IC], F16, tag="rb", name="rb")
                        nc.gpsimd.partition_broadcast(rbc, rr[0:1, :])
                        onrm = ot_pool.tile([DH, IC], F16, tag="on", name="on")
                        nc.gpsimd.tensor_tensor(onrm, of[0:DH, :], rbc, MUL)
                        for tp in range(2):  # two 256-token po tiles
                            po = st_pool.tile([128, 2 * IC], F32, tag="st", name="po")
                            for q in range(2):
                                tg = ic * 4 + tp * 2 + q
                                off = (tp * 2 + q) * 128
                                nc.tensor.matmul(
                                    po[:, q * IC:(q + 1) * IC],
                                    lhsT=onrm[:, off:off + 128], rhs=wout_sb,
                                    start=True, stop=True, skip_group_check=True)
                            osb = osb_pool.tile([128, 2 * IC], F16, tag="osb")
                            nc.vector.tensor_copy(osb, po)
                            r0 = bb * n + ic * IC + tp * 256
                            nc.sync.dma_start(
                                out=out[r0:r0 + 256, :].rearrange("(t p) d -> p t d", p=128),
                                in_=osb.rearrange("p (t d) -> p t d", t=2))
    nc.compile()
    return nc


def make_in_maps(query, pos_bias, Wq, Wk, Wv, Wout, n_cores=N_CORES):
    """Host-side sharding/layout prep. Head h -> core h."""
    query = np.asarray(query, dtype=np.float32)
    pos_bias = np.asarray(pos_bias, dtype=np.float32)
    Wq = np.asarray(Wq, dtype=np.float32)
    Wk = np.asarray(Wk, dtype=np.float32)
    Wv = np.asarray(Wv, dtype=np.float32)
    Wout = np.asarray(Wout, dtype=np.float32)

    b, n, d = query.shape
    qT = np.ascontiguousarray(query.reshape(b * n, d).T.astype(np.float16))
    wq_s = Wq * np.float32(SCALE)
    in_maps = []
    for h in range(n_cores):
        sl = slice(h * DH, (h + 1) * DH)
        in_maps.append({
            "qT": qT,
            "eb": np.ascontiguousarray(np.exp(pos_bias[h].T).astype(np.float16)),
            "wqk": np.ascontiguousarray(
                np.concatenate([wq_s[:, sl], Wk[:, sl]], axis=1).astype(np.float16)),
            "wv": np.ascontiguousarray(Wv[:, sl].astype(np.float16)),
            "wout": np.ascontiguousarray(Wout[sl, :].astype(np.float16)),
        })
    return in_maps


def run_device(in_maps, b=B, n=N, d=D, trace=False, **kw):
    nc = build_nc(b, n, d, n_cores=len(in_maps))
    return run_bass_kernel_spmd(nc, in_maps, list(range(len(in_maps))), trace=trace, **kw)


def assemble(results, b=B, n=N, d=D):
    acc = np.zeros((b * n, d), dtype=np.float32)
    for r in results:
        acc += r["out"]
    return acc.reshape(b, n, d)


def kernel(query, pos_bias, Wq, Wk, Wv, Wout):
    in_maps = make_in_maps(query, pos_bias, Wq, Wk, Wv, Wout)
    res = run_device(in_maps)
    return assemble(res.results)


# revision 12
# speedup vs baseline: 1.1872x; 1.1189x over previous
"""Multi-head self-attention with positional bias, sharded over 8 NeuronCores.

Sharding: head-parallel. Core h computes head h for all batches; the full
output is the sum of the 8 per-core partials (row-parallel Wout), reduced on
host.

Device kernel (per core), fp16 matmul inputs / fp32 PSUM accumulation:
  - projections: packed q|k weight [d, 128] gives one [128, 512]-psum chain
    per token chunk (q rows 0-63, k rows 64-127); v accumulates 16 token
    tiles side by side in one [128, 1024] psum tile.
  - scores are computed TRANSPOSED: ST[j, i] = k_j . q_i so exp's output is
    directly the layout the attention*V matmul needs.
  - the positional bias never touches the PE: host ships E = exp(bias^T) and
    the device computes P~ = exp(ST) * E with a 2x-mode fp16 DVE multiply.
  - softmax denominator: ones column appended to v; PV matmul row 64 then
    holds sum_j P~[j, i]. Normalization happens BEFORE the output projection
    (ot * recip[i], a broadcast fp16 multiply), so the Wout psum tiles DMA
    straight to DRAM with no extra engine pass.
"""

import numpy as np
from contextlib import ExitStack

import concourse.bass as bass
import concourse.bacc as bacc
import concourse.mybir as mybir
import concourse.tile as tile
from concourse.bass_utils import run_bass_kernel_spmd

HEADS = 8
DH = 64
B, N, D = 4, 2048, 512
SCALE = DH ** -0.5
N_CORES = 8

F32 = mybir.dt.float32
F16 = mybir.dt.float16
MUL = mybir.AluOpType.mult


def build_nc(b=B, n=N, d=D, n_cores=1):
    """Per-core Bass program (SPMD: per-head differences come in via inputs)."""
    assert b % 2 == 0 and n % 512 == 0 and d % 128 == 0
    T = b * n
    CC = d // 128        # contraction chunks for projections
    NJ = n // 128        # key tiles (j)
    IC = 512
    NIC = n // IC        # i-chunks of 512
    NIP = NIC // 2       # i-groups of 1024
    NPAIR = b // 2
    VW = DH + 1          # v block width (+1 ones column for denominator)

    nc = bacc.Bacc("TRN2", target_bir_lowering=False, debug=False,
                   num_devices=n_cores)
    qT = nc.declare_dram_parameter("qT", [d, T], F16, isOutput=False)
    eb = nc.declare_dram_parameter("eb", [n, n], F16, isOutput=False)
    wqk = nc.declare_dram_parameter("wqk", [d, 2 * DH], F16, isOutput=False)
    wv = nc.declare_dram_parameter("wv", [d, DH], F16, isOutput=False)
    wout = nc.declare_dram_parameter("wout", [DH, d], F16, isOutput=False)
    out = nc.declare_dram_parameter("out", [T, d], F16, isOutput=True)

    with ExitStack() as ctx:
        tc = ctx.enter_context(tile.TileContext(nc))

        const = ctx.enter_context(tc.tile_pool(name="const", bufs=1))
        qk_pool = ctx.enter_context(tc.tile_pool(name="qkT", bufs=1))
        v_pool = ctx.enter_context(tc.tile_pool(name="v", bufs=1))
        e_pool = ctx.enter_context(tc.tile_pool(name="ebias", bufs=1))
        ot_pool = ctx.enter_context(tc.tile_pool(name="otf", bufs=3))
        qt_pool = ctx.enter_context(tc.tile_pool(name="qt", bufs=6))
        p_pool = ctx.enter_context(tc.tile_pool(name="pexp", bufs=4))
        pr_pool = ctx.enter_context(tc.tile_pool(name="prod", bufs=4))
        osb_pool = ctx.enter_context(tc.tile_pool(name="osb", bufs=4))
        # PSUM: st_pool holds score tiles, projection accumulators and output
        # po tiles (all [128, 1024] f32 = 2 banks); ots holds PV accumulators.
        st_pool = ctx.enter_context(tc.tile_pool(name="st", bufs=2, space="PSUM"))
        ots_pool = ctx.enter_context(tc.tile_pool(name="ots", bufs=4, space="PSUM"))

        zbias = const.tile([128, 1], F32, tag="zbias")
        nc.vector.memset(zbias, 0.0)
        ones16 = const.tile([128, 16], F16, tag="ones16")
        nc.vector.memset(ones16, 1.0)

        wqk_sb = const.tile([128, CC, 2 * DH], F16, tag="wqk")
        nc.sync.dma_start(out=wqk_sb, in_=wqk[:, :].rearrange("(c p) e -> p c e", p=128))
        wv_sb = const.tile([128, CC, DH], F16, tag="wv")
        nc.sync.dma_start(out=wv_sb, in_=wv[:, :].rearrange("(c p) e -> p c e", p=128))
        wout_sb = const.tile([DH, d], F16, tag="wout")
        nc.sync.dma_start(out=wout_sb, in_=wout[:, :])

        # exp(bias^T), staged whole: E_sb[jt][p, i] = exp(bias[i, jt*128+p])
        e_sb = []
        for jt in range(NJ):
            t = e_pool.tile([128, n], F16, tag=f"eb{jt}", name=f"eb{jt}")
            nc.sync.dma_start(out=t, in_=eb[jt * 128:(jt + 1) * 128, :])
            e_sb.append(t)

        qT_sb = [qk_pool.tile([DH, n], F16, tag=f"qT{bb}", name=f"qT{bb}") for bb in range(b)]
        kT_sb = [qk_pool.tile([DH, n], F16, tag=f"kT{bb}", name=f"kT{bb}") for bb in range(b)]
        v_sb = [v_pool.tile([128, NJ * VW], F16, tag=f"v{bb}", name=f"v{bb}") for bb in range(b)]
        for bb in range(b):
            ones_cols = v_sb[bb].rearrange("p (t w) -> p t w", w=VW)[:, :, DH:VW]
            nc.vector.tensor_copy(ones_cols, ones16[:, 0:NJ].rearrange("p (t o) -> p t o", o=1))



        # ---------------- projections (per batch) ----------------
        for bb in range(b):
            qt_c = []
            for c in range(CC):
                t = qt_pool.tile([128, n], F16, tag="qt", name=f"qt{bb}_{c}")
                nc.sync.dma_start(out=t, in_=qT[c * 128:(c + 1) * 128, bb * n:(bb + 1) * n])
                qt_c.append(t)
            # q|k packed: psum rows 0-63 = q^T, 64-127 = k^T
            for hh in range(n // (2 * IC)):
                ps = st_pool.tile([128, 2 * IC], F32, tag="st", name=f"pqk{bb}_{hh}")
                for half in range(2):
                    cols = slice(half * IC, (half + 1) * IC)
                    acols = slice(hh * 2 * IC + half * IC, hh * 2 * IC + (half + 1) * IC)
                    for c in range(CC):
                        nc.tensor.matmul(ps[:, cols], lhsT=wqk_sb[:, c, :],
                                         rhs=qt_c[c][:, acols],
                                         start=(c == 0), stop=(c == CC - 1),
                                         skip_group_check=True)
                dcols = slice(hh * 2 * IC, (hh + 1) * 2 * IC)
                nc.vector.tensor_copy(qT_sb[bb][:, dcols], ps[0:DH, :])
                nc.vector.tensor_copy(kT_sb[bb][:, dcols], ps[DH:128, :])
            # v: 16 token tiles side by side in one [128, 1024] psum tile
            psv = st_pool.tile([128, 2 * IC], F32, tag="st", name=f"pv{bb}")
            for tt in range(NJ):
                for c in range(CC):
                    nc.tensor.matmul(psv[:, tt * DH:(tt + 1) * DH],
                                     lhsT=qt_c[c][:, tt * 128:(tt + 1) * 128],
                                     rhs=wv_sb[:, c, :],
                                     start=(c == 0), stop=(c == CC - 1),
                                     skip_group_check=True)
            vdst = v_sb[bb].rearrange("p (t w) -> p t w", w=VW)[:, :, 0:DH]
            nc.vector.tensor_copy(vdst, psv.rearrange("p (t e) -> p t e", e=DH))

        # ---------------- scores + softmax + P~^T V + out-proj ----------------
        # Software-pipelined emission: engines dispatch in-order with a
        # single-slot wait queue, so PV matmuls are emitted DEPTH steps after
        # their qk/exp/prod chain, and the block epilogue (evac, reciprocal,
        # normalize, Wout matmuls, store) is spread into the next block's
        # steps. This keeps the PE/Act queues free of head-of-line stalls.
        exp_fn = mybir.ActivationFunctionType.Exp
        PROD_POOL_JTS = frozenset(range(3, NJ, 4))  # DVE/Pool work split
        DEPTH, POOL_DEPTH = 2, 3

        steps = [(ip, pair, jt, lb)
                 for ip in range(NIP) for pair in range(NPAIR)
                 for jt in range(NJ) for lb in range(2)]
        SPB = NJ * 2  # steps per (ip, pair) block

        ot_ps_blk = {}     # block index -> {(lb, il): psum tile}
        pv_q = []          # (release_step, fn)
        extra_q = []       # (release_step, fn)

        def emit_pv(blk, pair, jt, lb, prod):
            def fn():
                bb = 2 * pair + lb
                for il in range(2):
                    nc.tensor.matmul(
                        ot_ps_blk[blk][(lb, il)],
                        lhsT=v_sb[bb][:, jt * VW:jt * VW + VW],
                        rhs=prod[:, il * IC:(il + 1) * IC],
                        start=(jt == 0), stop=(jt == NJ - 1))
            return fn

        def emit_evac(blk, ip, pair, lb, il):
            def fn():
                bb = 2 * pair + lb
                of = ot_pool.tile([VW, IC], F16, tag="of", name="of")
                nc.vector.tensor_copy(of, ot_ps_blk[blk][(lb, il)])
                rr = ot_pool.tile([1, IC], F16, tag="rr", name="rr")
                with nc.allow_low_precision("fp16 softmax denom reciprocal"):
                    nc.vector.reciprocal(rr, of[DH:VW, :])
                rbc = ot_pool.tile([DH, IC], F16, tag="rb", name="rb")
                nc.gpsimd.partition_broadcast(rbc, rr[0:1, :])
                onrm = ot_pool.tile([DH, IC], F16, tag="on", name="on")
                nc.gpsimd.tensor_tensor(onrm, of[0:DH, :], rbc, MUL)
                onorm_blk[(blk, lb, il)] = onrm
            return fn

        def emit_po(blk, ip, pair, lb, il, tp):
            def fn():
                bb = 2 * pair + lb
                ic = ip * 2 + il
                onrm = onorm_blk[(blk, lb, il)]
                po = st_pool.tile([128, 2 * IC], F32, tag="st", name="po")
                for q in range(2):
                    off = (tp * 2 + q) * 128
                    nc.tensor.matmul(
                        po[:, q * IC:(q + 1) * IC],
                        lhsT=onrm[:, off:off + 128], rhs=wout_sb,
                        start=True, stop=True, skip_group_check=True)
                osb = osb_pool.tile([128, 2 * IC], F16, tag="osb")
                nc.vector.tensor_copy(osb, po)
                r0 = bb * n + ic * IC + tp * 256
                nc.sync.dma_start(
                    out=out[r0:r0 + 256, :].rearrange("(t p) d -> p t d", p=128),
                    in_=osb.rearrange("p (t d) -> p t d", t=2))
            return fn

        onorm_blk = {}
        n_steps = len(steps)
        for s in range(n_steps + DEPTH + 2):
            if s < n_steps:
                ip, pair, jt, lb = steps[s]
                blk = s // SPB
                if s % SPB == 0:
                    ot_ps_blk[blk] = {
                        (l2, i2): ots_pool.tile([VW, IC], F32, tag="ot", name="otp")
                        for l2 in range(2) for i2 in range(2)}
                bb = 2 * pair + lb
                st = st_pool.tile([128, 2 * IC], F32, tag="st", name="st")
                for il in range(2):
                    ic = ip * 2 + il
                    nc.tensor.matmul(
                        st[:, il * IC:(il + 1) * IC],
                        lhsT=kT_sb[bb][:, jt * 128:(jt + 1) * 128],
                        rhs=qT_sb[bb][:, ic * IC:(ic + 1) * IC],
                        start=True, stop=True, skip_group_check=True)
                pexp = p_pool.tile([128, 2 * IC], F16, tag="pexp")
                nc.scalar.activation(pexp, st, exp_fn, bias=zbias)
                prod = pr_pool.tile([128, 2 * IC], F16, tag="prod")
                on_pool = jt in PROD_POOL_JTS
                peng = nc.gpsimd if on_pool else nc.vector
                peng.tensor_tensor(
                    prod, pexp, e_sb[jt][:, ip * 2 * IC:(ip + 1) * 2 * IC], MUL)
                rel = s + (POOL_DEPTH if on_pool else DEPTH)
                pv_q.append((rel, emit_pv(blk, pair, jt, lb, prod)))
                if s % SPB == SPB - 1:  # schedule this block's epilogue
                    base = s + DEPTH + 1
                    k = 0
                    for l2 in range(2):
                        for i2 in range(2):
                            extra_q.append((base + k, emit_evac(blk, ip, pair, l2, i2)))
                            for tp in range(2):
                                extra_q.append((base + 4 + 2 * k + tp,
                                                emit_po(blk, ip, pair, l2, i2, tp)))
                            k += 1
            for q in (pv_q, extra_q):
                ready = [f for r, f in q if r <= s]
                q[:] = [(r, f) for r, f in q if r > s]
                for f in ready:
                    f()
    nc.compile()
    return nc


def make_in_maps(query, pos_bias, Wq, Wk, Wv, Wout, n_cores=N_CORES):
    """Host-side sharding/layout prep. Head h -> core h."""
    query = np.asarray(query, dtype=np.float32)
    pos_bias = np.asarray(pos_bias, dtype=np.float32)
    Wq = np.asarray(Wq, dtype=np.float32)
    Wk = np.asarray(Wk, dtype=np.float32)
    Wv = np.asarray(Wv, dtype=np.float32)
    Wout = np.asarray(Wout, dtype=np.float32)

    b, n, d = query.shape
    qT = np.ascontiguousarray(query.reshape(b * n, d).T.astype(np.float16))
    wq_s = Wq * np.float32(SCALE)
    in_maps = []
    for h in range(n_cores):
        sl = slice(h * DH, (h + 1) * DH)
        in_maps.append({
            "qT": qT,
            "eb": np.ascontiguousarray(np.exp(pos_bias[h].T).astype(np.float16)),
            "wqk": np.ascontiguousarray(
                np.concatenate([wq_s[:, sl], Wk[:, sl]], axis=1).astype(np.float16)),
            "wv": np.ascontiguousarray(Wv[:, sl].astype(np.float16)),
            "wout": np.ascontiguousarray(Wout[sl, :].astype(np.float16)),
        })
    return in_maps


def run_device(in_maps, b=B, n=N, d=D, trace=False, **kw):
    nc = build_nc(b, n, d, n_cores=len(in_maps))
    return run_bass_kernel_spmd(nc, in_maps, list(range(len(in_maps))), trace=trace, **kw)


def assemble(results, b=B, n=N, d=D):
    acc = np.zeros((b * n, d), dtype=np.float32)
    for r in results:
        acc += r["out"]
    return acc.reshape(b, n, d)


def kernel(query, pos_bias, Wq, Wk, Wv, Wout):
    in_maps = make_in_maps(query, pos_bias, Wq, Wk, Wv, Wout)
    res = run_device(in_maps)
    return assemble(res.results)


# revision 18
# speedup vs baseline: 1.2795x; 1.0778x over previous
"""Multi-head self-attention with positional bias, sharded over 8 NeuronCores.

Sharding: head-parallel. Core h computes head h for all batches; the full
output is the sum of the 8 per-core partials (row-parallel Wout), reduced on
host.

Device kernel (per core), fp16 matmul inputs / fp32 PSUM accumulation:
  - projections: packed q|k weight [d, 128] gives one [128, 512]-psum chain
    per token chunk (q rows 0-63, k rows 64-127); v accumulates 16 token
    tiles side by side in one [128, 1024] psum tile.
  - scores are computed TRANSPOSED: ST[j, i] = k_j . q_i so exp's output is
    directly the layout the attention*V matmul needs.
  - the positional bias never touches the PE: host ships E = exp(bias^T) and
    the device computes P~ = exp(ST) * E with a 2x-mode fp16 DVE multiply.
  - softmax denominator: ones column appended to v; PV matmul row 64 then
    holds sum_j P~[j, i]. Normalization happens BEFORE the output projection
    (ot * recip[i], a broadcast fp16 multiply), so the Wout psum tiles DMA
    straight to DRAM with no extra engine pass.
"""

import numpy as np
from contextlib import ExitStack

import concourse.bass as bass
import concourse.bacc as bacc
import concourse.mybir as mybir
import concourse.tile as tile
from concourse.bass_utils import run_bass_kernel_spmd

HEADS = 8
DH = 64
B, N, D = 4, 2048, 512
SCALE = DH ** -0.5
N_CORES = 8

F32 = mybir.dt.float32
F16 = mybir.dt.float16
MUL = mybir.AluOpType.mult


def build_nc(b=B, n=N, d=D, n_cores=1):
    """Per-core Bass program (SPMD: per-head differences come in via inputs)."""
    assert b % 2 == 0 and n % 512 == 0 and d % 128 == 0
    T = b * n
    CC = d // 128        # contraction chunks for projections
    NJ = n // 128        # key tiles (j)
    IC = 512
    NIC = n // IC        # i-chunks of 512
    NIP = NIC // 2       # i-groups of 1024
    NPAIR = b // 2
    VW = DH + 1          # v block width (+1 ones column for denominator)

    nc = bacc.Bacc("TRN2", target_bir_lowering=False, debug=False,
                   num_devices=n_cores)
    qT = nc.declare_dram_parameter("qT", [d, T], F16, isOutput=False)
    eb = nc.declare_dram_parameter("eb", [n, n], F16, isOutput=False)
    wqk = nc.declare_dram_parameter("wqk", [d, 2 * DH], F16, isOutput=False)
    wv = nc.declare_dram_parameter("wv", [d, DH], F16, isOutput=False)
    wout = nc.declare_dram_parameter("wout", [DH, d], F16, isOutput=False)
    out = nc.declare_dram_parameter("out", [T, d], F16, isOutput=True)

    with ExitStack() as ctx:
        tc = ctx.enter_context(tile.TileContext(nc))

        const = ctx.enter_context(tc.tile_pool(name="const", bufs=1))
        qk_pool = ctx.enter_context(tc.tile_pool(name="qkT", bufs=1))
        v_pool = ctx.enter_context(tc.tile_pool(name="v", bufs=1))
        e_pool = ctx.enter_context(tc.tile_pool(name="ebias", bufs=1))
        ot_pool = ctx.enter_context(tc.tile_pool(name="otf", bufs=3))
        qt_pool = ctx.enter_context(tc.tile_pool(name="qt", bufs=6))
        p_pool = ctx.enter_context(tc.tile_pool(name="pexp", bufs=4))
        pr_pool = ctx.enter_context(tc.tile_pool(name="prod", bufs=4))
        osb_pool = ctx.enter_context(tc.tile_pool(name="osb", bufs=4))
        # PSUM: st_pool holds score tiles, projection accumulators and output
        # po tiles (all [128, 1024] f32 = 2 banks); ots holds PV accumulators.
        st_pool = ctx.enter_context(tc.tile_pool(name="st", bufs=2, space="PSUM"))
        ots_pool = ctx.enter_context(tc.tile_pool(name="ots", bufs=4, space="PSUM"))

        zbias = const.tile([128, 1], F32, tag="zbias")
        nc.vector.memset(zbias, 0.0)
        ones16 = const.tile([128, 16], F16, tag="ones16")
        nc.vector.memset(ones16, 1.0)

        wqk_sb = const.tile([128, CC, 2 * DH], F16, tag="wqk")
        nc.sync.dma_start(out=wqk_sb, in_=wqk[:, :].rearrange("(c p) e -> p c e", p=128))
        wv_sb = const.tile([128, CC, DH], F16, tag="wv")
        nc.sync.dma_start(out=wv_sb, in_=wv[:, :].rearrange("(c p) e -> p c e", p=128))
        wout_sb = const.tile([DH, d], F16, tag="wout")
        nc.sync.dma_start(out=wout_sb, in_=wout[:, :])

        qT_sb = [qk_pool.tile([DH, n], F16, tag=f"qT{bb}", name=f"qT{bb}") for bb in range(b)]
        kT_sb = [qk_pool.tile([DH, n], F16, tag=f"kT{bb}", name=f"kT{bb}") for bb in range(b)]
        v_sb = [v_pool.tile([128, NJ * VW], F16, tag=f"v{bb}", name=f"v{bb}") for bb in range(b)]
        for bb in range(b):
            ones_cols = v_sb[bb].rearrange("p (t w) -> p t w", w=VW)[:, :, DH:VW]
            nc.vector.tensor_copy(ones_cols, ones16[:, 0:NJ].rearrange("p (t o) -> p t o", o=1))



        # ---------------- projections (per batch) ----------------
        # DMA order matters (SP queue + DMA engines are serial): qt for the
        # first two batches goes out first so projections start immediately;
        # the E = exp(bias^T) staging streams behind it.
        e_sb = []

        def load_e_tiles():
            for jt in range(NJ):
                t = e_pool.tile([128, n], F16, tag=f"eb{jt}", name=f"eb{jt}")
                nc.sync.dma_start(out=t, in_=eb[jt * 128:(jt + 1) * 128, :])
                e_sb.append(t)

        for bb in range(b):
            if bb == 2:
                load_e_tiles()
            qt_c = []
            for c in range(CC):
                t = qt_pool.tile([128, n], F16, tag="qt", name=f"qt{bb}_{c}")
                nc.sync.dma_start(out=t, in_=qT[c * 128:(c + 1) * 128, bb * n:(bb + 1) * n])
                qt_c.append(t)
            # q|k packed: psum rows 0-63 = q^T, 64-127 = k^T
            for hh in range(n // (2 * IC)):
                ps = st_pool.tile([128, 2 * IC], F32, tag="st", name=f"pqk{bb}_{hh}")
                for half in range(2):
                    cols = slice(half * IC, (half + 1) * IC)
                    acols = slice(hh * 2 * IC + half * IC, hh * 2 * IC + (half + 1) * IC)
                    for c in range(CC):
                        nc.tensor.matmul(ps[:, cols], lhsT=wqk_sb[:, c, :],
                                         rhs=qt_c[c][:, acols],
                                         start=(c == 0), stop=(c == CC - 1),
                                         skip_group_check=True)
                dcols = slice(hh * 2 * IC, (hh + 1) * 2 * IC)
                nc.vector.tensor_copy(qT_sb[bb][:, dcols], ps[0:DH, :])
                nc.vector.tensor_copy(kT_sb[bb][:, dcols], ps[DH:128, :])
            # v: 16 token tiles side by side in one [128, 1024] psum tile
            psv = st_pool.tile([128, 2 * IC], F32, tag="st", name=f"pv{bb}")
            for tt in range(NJ):
                for c in range(CC):
                    nc.tensor.matmul(psv[:, tt * DH:(tt + 1) * DH],
                                     lhsT=qt_c[c][:, tt * 128:(tt + 1) * 128],
                                     rhs=wv_sb[:, c, :],
                                     start=(c == 0), stop=(c == CC - 1),
                                     skip_group_check=True)
            vdst = v_sb[bb].rearrange("p (t w) -> p t w", w=VW)[:, :, 0:DH]
            nc.vector.tensor_copy(vdst, psv.rearrange("p (t e) -> p t e", e=DH))

        # ---------------- scores + softmax + P~^T V + out-proj ----------------
        # Software-pipelined emission: engines dispatch in-order with a
        # single-slot wait queue, so PV matmuls are emitted DEPTH steps after
        # their qk/exp/prod chain, and the block epilogue (evac, reciprocal,
        # normalize, Wout matmuls, store) is spread into the next block's
        # steps. This keeps the PE/Act queues free of head-of-line stalls.
        exp_fn = mybir.ActivationFunctionType.Exp
        PROD_POOL_JTS = frozenset((3, 5, 7, 11, 13, 15))  # DVE/Pool work split
        DEPTH, POOL_DEPTH = 2, 5

        steps = [(ip, pair, jt, lb)
                 for ip in range(NIP) for pair in range(NPAIR)
                 for jt in range(NJ) for lb in range(2)]
        SPB = NJ * 2  # steps per (ip, pair) block

        ot_ps_blk = {}     # block index -> {(lb, il): psum tile}
        pv_q = []          # (release_step, fn)
        extra_q = []       # (release_step, fn)

        def emit_pv(blk, pair, jt, lb, prod):
            def fn():
                bb = 2 * pair + lb
                for il in range(2):
                    nc.tensor.matmul(
                        ot_ps_blk[blk][(lb, il)],
                        lhsT=v_sb[bb][:, jt * VW:jt * VW + VW],
                        rhs=prod[:, il * IC:(il + 1) * IC],
                        start=(jt == 0), stop=(jt == NJ - 1),
                        skip_group_check=True)
            return fn

        def emit_evac(blk, ip, pair, lb, il):
            def fn():
                of = ot_pool.tile([VW, IC], F16, tag="of", name="of")
                nc.vector.tensor_copy(of, ot_ps_blk[blk][(lb, il)])
                rr = ot_pool.tile([1, IC], F16, tag="rr", name="rr")
                with nc.allow_low_precision("fp16 softmax denom reciprocal"):
                    nc.vector.reciprocal(rr, of[DH:VW, :])
                of_blk[(blk, lb, il)] = (of, rr)
            return fn

        def emit_norm(blk, lb, il):
            def fn():
                of, rr = of_blk[(blk, lb, il)]
                rbc = ot_pool.tile([DH, IC], F16, tag="rb", name="rb")
                nc.gpsimd.partition_broadcast(rbc, rr[0:1, :])
                onrm = ot_pool.tile([DH, IC], F16, tag="on", name="on")
                nc.vector.tensor_tensor(onrm, of[0:DH, :], rbc, MUL)
                onorm_blk[(blk, lb, il)] = onrm
            return fn

        def emit_po(blk, ip, pair, lb, il, tp):
            def fn():
                bb = 2 * pair + lb
                ic = ip * 2 + il
                onrm = onorm_blk[(blk, lb, il)]
                po = st_pool.tile([128, 2 * IC], F32, tag="st", name="po")
                for q in range(2):
                    off = (tp * 2 + q) * 128
                    nc.tensor.matmul(
                        po[:, q * IC:(q + 1) * IC],
                        lhsT=onrm[:, off:off + 128], rhs=wout_sb,
                        start=True, stop=True, skip_group_check=True)
                osb = osb_pool.tile([128, 2 * IC], F16, tag="osb")
                nc.vector.tensor_copy(osb, po)
                r0 = bb * n + ic * IC + tp * 256
                nc.sync.dma_start(
                    out=out[r0:r0 + 256, :].rearrange("(t p) d -> p t d", p=128),
                    in_=osb.rearrange("p (t d) -> p t d", t=2))
            return fn

        onorm_blk = {}
        of_blk = {}
        n_steps = len(steps)
        for s in range(n_steps + POOL_DEPTH + 14):
            if s < n_steps:
                ip, pair, jt, lb = steps[s]
                blk = s // SPB
                if s % SPB == 0:
                    ot_ps_blk[blk] = {
                        (l2, i2): ots_pool.tile([VW, IC], F32, tag="ot", name="otp")
                        for l2 in range(2) for i2 in range(2)}
                bb = 2 * pair + lb
                st = st_pool.tile([128, 2 * IC], F32, tag="st", name="st")
                for il in range(2):
                    ic = ip * 2 + il
                    nc.tensor.matmul(
                        st[:, il * IC:(il + 1) * IC],
                        lhsT=kT_sb[bb][:, jt * 128:(jt + 1) * 128],
                        rhs=qT_sb[bb][:, ic * IC:(ic + 1) * IC],
                        start=True, stop=True, skip_group_check=True)
                pexp = p_pool.tile([128, 2 * IC], F16, tag="pexp")
                nc.scalar.activation(pexp, st, exp_fn, bias=zbias)
                prod = pr_pool.tile([128, 2 * IC], F16, tag="prod")
                on_pool = jt in PROD_POOL_JTS
                peng = nc.gpsimd if on_pool else nc.vector
                peng.tensor_tensor(
                    prod, pexp, e_sb[jt][:, ip * 2 * IC:(ip + 1) * 2 * IC], MUL)
                rel = s + (POOL_DEPTH if on_pool else DEPTH)
                pv_q.append((rel, emit_pv(blk, pair, jt, lb, prod)))
                if s % SPB == SPB - 1:  # schedule this block's epilogue
                    base = s + POOL_DEPTH + 1  # after the block's last PV
                    k = 0
                    for l2 in range(2):
                        for i2 in range(2):
                            extra_q.append((base + 2 * k, emit_evac(blk, ip, pair, l2, i2)))
                            extra_q.append((base + 2 * k + 1, emit_norm(blk, l2, i2)))
                            for tp in range(2):
                                extra_q.append((base + 4 + 2 * k + tp,
                                                emit_po(blk, ip, pair, l2, i2, tp)))
                            k += 1
            for q in (pv_q, extra_q):
                ready = [f for r, f in q if r <= s]
                q[:] = [(r, f) for r, f in q if r > s]
                for f in ready:
                    f()
    nc.compile()
    return nc


def make_in_maps(query, pos_bias, Wq, Wk, Wv, Wout, n_cores=N_CORES):
    """Host-side sharding/layout prep. Head h -> core h."""
    query = np.asarray(query, dtype=np.float32)
    pos_bias = np.asarray(pos_bias, dtype=np.float32)
    Wq = np.asarray(Wq, dtype=np.float32)
    Wk = np.asarray(Wk, dtype=np.float32)
    Wv = np.asarray(Wv, dtype=np.float32)
    Wout = np.asarray(Wout, dtype=np.float32)

    b, n, d = query.shape
    qT = np.ascontiguousarray(query.reshape(b * n, d).T.astype(np.float16))
    wq_s = Wq * np.float32(SCALE)
    in_maps = []
    for h in range(n_cores):
        sl = slice(h * DH, (h + 1) * DH)
        in_maps.append({
            "qT": qT,
            "eb": np.ascontiguousarray(np.exp(pos_bias[h].T).astype(np.float16)),
            "wqk": np.ascontiguousarray(
                np.concatenate([wq_s[:, sl], Wk[:, sl]], axis=1).astype(np.float16)),
            "wv": np.ascontiguousarray(Wv[:, sl].astype(np.float16)),
            "wout": np.ascontiguousarray(Wout[sl, :].astype(np.float16)),
        })
    return in_maps


def run_device(in_maps, b=B, n=N, d=D, trace=False, **kw):
    nc = build_nc(b, n, d, n_cores=len(in_maps))
    return run_bass_kernel_spmd(nc, in_maps, list(range(len(in_maps))), trace=trace, **kw)


def assemble(results, b=B, n=N, d=D):
    acc = np.zeros((b * n, d), dtype=np.float32)
    for r in results:
        acc += r["out"]
    return acc.reshape(b, n, d)


def kernel(query, pos_bias, Wq, Wk, Wv, Wout):
    in_maps = make_in_maps(query, pos_bias, Wq, Wk, Wv, Wout)
    res = run_device(in_maps)
    return assemble(res.results)
